# revision 45
# baseline (speedup 1.0000x reference)
"""Trainium2 Bass kernel for nn_AttentionBlock (B=4, C=64, H=W=64, INTER=8).

Sharding: 8 cores = 4 batches x 2 query-halves. Each core computes, for its
batch b and its half of the query pixels (n), the full attention output
gamma * (V @ softmax(Q^T K)^T) + x over all m=4096 keys.

SPMD uniformity trick: the host permutes each core's pixel columns so that
columns [0, 2048) are the core's OWN query half and [2048, 4096) are the
other half. Attention is permutation-invariant over keys, so every core runs
the identical program on differently-permuted data.

Design notes (the chip power-throttles under sustained load, pinning the
PE near 1.2 GHz and making the kernel PE-throughput-bound; every change
below removes PE cycles or overlaps engines):
  1. kq setup: COLUMN-TILED matmuls (4 concurrent tiles of M=8 at
     tile_position=(0,32i)) write k (and q) of each 512-pixel chunk
     directly into partition groups {0,32,64,96}+0..7 of one PSUM bank;
     a single fat DVE cast lands them in kall/qall SBUF, replacing a long
     per-group copy chain.
  2. Energy matmuls are 4-way ROW-TILED (tile_position=(32i,0), K=8 of
     32): up to 4 concurrent m-block matmuls per group burst.
  3. exp groups of GRP=3 m-blocks (3 PSUM banks x 2 bufs): 44 ACT
     instructions amortize the ~400ns/instr ACT overhead. AV matmuls lag
     one group behind exp so the PE runs ahead of the ACT.
  4. AV (vT_aug^T @ expE) accumulates out_aug[65, 512] per chunk; row 64
     (vT ones column) is the softmax denominator.
  5. gamma*bv is folded into the residual input on the host (out =
     gamma*(V@A/d) + gamma*bv + x, since sum_m A[n,m] = d[n]), so Wv has
     no bias row and vt matmuls contract over K=64.
  6. The natural_log_exp_and_others ACT table set (ln+exp) is preloaded
     once, so the tail 1/x = exp(-ln(x)) trick causes no table reloads.
     Mid-stream chunks use DVE reciprocal (slow but off the critical
     engines); their epilogue starts with a PSUM->SBUF copy-out so the
     oa bank frees immediately. The tail broadcasts the reciprocal row
     with a PE matmul (ones64^T @ rec) instead of the gpsimd queue.

Hard-won constraints (hangs/verifier): PSUM engine reads must start at a
32-aligned partition; two matmuls with DIFFERENT tile_positions must not
write the same PSUM bank (device hang); DVE tensor ops may read at most
one PSUM operand.

No max-subtraction is needed in softmax: |energy| <~ 15 for this problem's
fixed input distribution, well within fp32 exp range.
"""

import os
import sys
import types
import numpy as np
import ml_dtypes


def _ensure_ntff_hook_importable():
    """bass_utils imports antenv.axon_hooks when tracing is requested via
    BASS_TRACE; some images lack that module. Provide it (backed by the
    ctypes hook from trn_boot when available, else a None hook, which
    bass_utils handles by skipping the trace)."""
    try:
        import antenv.axon_hooks  # noqa: F401
        return
    except ImportError:
        pass
    hook = None
    try:
        from trn_agent_boot.trn_boot import _ntff_profile_via_ctypes
        so = "/opt/axon/libaxon_pjrt.so"
        if os.path.exists(so):
            hook = _ntff_profile_via_ctypes(so)
    except Exception:
        hook = None
    mod = types.ModuleType("antenv.axon_hooks")
    mod.get_axon_ntff_profile_hook = lambda: hook
    sys.modules["antenv.axon_hooks"] = mod

B, C, H, W = 4, 64, 64, 64
N = H * W              # 4096 pixels
NHALF = N // 2         # 2048 query pixels per core
INTER = C // 8         # 8
NCORES = 8
MBLK = 128             # m-block (PSUM partition tile)
NCHUNK = 512           # query-chunk (PSUM bank free size)
NJ = N // MBLK         # 32 m-blocks
NT = NHALF // NCHUNK   # 4 query chunks
GRP = int(os.environ.get("KGRP", "3"))      # m-blocks per exp instruction
NGRP = (NJ + GRP - 1) // GRP                # exp groups per chunk
ROWTILE = int(os.environ.get("KROWTILE", "1"))  # 4-way PE row tiling
KFUSE = int(os.environ.get("KFUSE", "0"))   # fused k+q bursts: rejected by
# the BIR verifier (matmul operands must start 32-partition-aligned, and
# fusing puts q at base 32i+8); kept for reference
ACT_SET_LN_EXP = 6     # act_info.json index of natural_log_exp_and_others

_compiled = {}
LAST_RESULT = None


def _build():
    import concourse.bacc as bacc
    import concourse.mybir as mybir
    from concourse.tile import TileContext

    dt = mybir.dt
    f32, bf16 = dt.float32, dt.bfloat16
    EXP = mybir.ActivationFunctionType.Exp
    LN = mybir.ActivationFunctionType.Ln

    nc = bacc.Bacc("TRN2", target_bir_lowering=False, debug=False,
                   num_devices=NCORES)

    # host-prepped inputs (see kernel() below)
    xbh = nc.dram_tensor("xbh", [130, NHALF], bf16, kind="ExternalInput").ap()
    xres = nc.dram_tensor("xres", [C, NHALF], f32, kind="ExternalInput").ap()
    wqk = nc.dram_tensor("wqk", [C + 1, 16], bf16, kind="ExternalInput").ap()
    wv = nc.dram_tensor("wv_", [C, C], bf16, kind="ExternalInput").ap()
    out = nc.dram_tensor("out", [C, NHALF], f32, kind="ExternalOutput").ap()

    with TileContext(nc) as tc:
        with tc.tile_pool(name="const", bufs=1) as cp, \
             tc.tile_pool(name="eps", bufs=2, space="PSUM") as eps, \
             tc.tile_pool(name="ops", bufs=8 - 2 * GRP,
                          space="PSUM") as ops, \
             tc.tile_pool(name="work", bufs=3) as wp, \
             tc.tile_pool(name="fin", bufs=2) as fp:

            # Preload the ln+exp activation-table set so neither the exp
            # stream nor the tail 1/x = exp(-ln x) forces a table switch.
            nc.scalar.add_instruction(mybir.InstLoadActFuncSet(
                name=nc.scalar.bass.get_next_instruction_name(),
                act_func_set_id=ACT_SET_LN_EXP, ins=[], outs=[]))

            # DMA issue order matters: the first kq matmul needs xqo piece
            # one + wqk; wv is needed by the first vt group shortly after;
            # xres only by the chunk-0 epilogue.
            # wqk+wv go out on the gpsimd DGE queue so they transfer in
            # parallel with the x chunk on the sync queue (the serial DMA
            # chain otherwise delays the first kq matmul by ~1us)
            wqk_t = cp.tile([C + 1, 16], bf16, tag="wqk", name="wqk_t")
            nc.gpsimd.dma_start(out=wqk_t[:, :], in_=wqk)
            wv_t = cp.tile([C, C], bf16, tag="wv", name="wv_t")
            nc.gpsimd.dma_start(out=wv_t[:, :], in_=wv)
            xqo = cp.tile([C + 1, NHALF], bf16, tag="xqo", name="xqo")
            nc.sync.dma_start(out=xqo[:, 0:NCHUNK], in_=xbh[0:C + 1, 0:NCHUNK])
            nc.sync.dma_start(out=xqo[:, NCHUNK:], in_=xbh[0:C + 1, NCHUNK:])
            xqt = cp.tile([C + 1, NHALF], bf16, tag="xqt", name="xqt")
            nc.sync.dma_start(out=xqt[:, :], in_=xbh[C + 1:2 * C + 2, :])
            xr_t = cp.tile([C, NHALF], f32, tag="xr", name="xr_t")
            nc.sync.dma_start(out=xr_t[:, :], in_=xres)
            ones64 = cp.tile([1, C], bf16, tag="ones64", name="ones64")
            nc.vector.memset(ones64[:, :], 1.0)

            # kall/qall: k (q) of every 512-pixel chunk replicated across
            # partition groups {0,32,64,96}+0..7 by COLUMN-TILED kq matmuls
            # (4 concurrent tiles, tile_position=(0,32i)), so a single fat
            # PSUM->SBUF cast replaces the whole per-group copy chain.
            # Energy lhsT for m-block j: kall[32(j%4)+0..8,
            # 512(j//4)+128(j%4) ..+128]; rhs: qall[32i+0..8, chunk].
            # kqall: per 512-pixel chunk window, partitions 32i+0..7 hold
            # that chunk's k and 32i+8..15 its q (all four groups), written
            # by ONE fused column-tiled burst (M=16 at tile_position
            # (0,32i)) and ONE fat cast. With KFUSE=0, separate k/q tiles.
            kqall = cp.tile([128, N], bf16, tag="kqall", name="kqall")
            if not KFUSE:
                qall = cp.tile([128, NHALF], bf16, tag="qall", name="qall")
            vt = cp.tile([128, NJ * (C + 1)], bf16, tag="vt", name="vt")
            vt3 = vt.rearrange("p (j c) -> p j c", c=C + 1)
            nc.vector.memset(vt3[:, :, C], 1.0)

            def emit_kq(kc):
                """kq chunk kc (0-3 own half, 4-7 other half)."""
                own = kc < NT
                srct = xqo if own else xqt
                t = kc % NT
                rhs = srct[:, NCHUNK * t:NCHUNK * (t + 1)]
                kp = ops.tile([128, NCHUNK], f32, tag="o", name="kp")
                m = 2 * INTER if KFUSE else INTER
                for i in range(4):
                    nc.tensor.matmul(kp[32 * i:32 * i + m, :],
                                     wqk_t[:, 0:m], rhs,
                                     start=True, stop=True,
                                     tile_position=(0, 32 * i))
                nc.vector.tensor_copy(
                    kqall[:, NCHUNK * kc:NCHUNK * (kc + 1)], kp[:, :])
                if own and not KFUSE:
                    qp = ops.tile([128, NCHUNK], f32, tag="o", name="qp")
                    for i in range(4):
                        nc.tensor.matmul(qp[32 * i:32 * i + INTER, :],
                                         wqk_t[:, INTER:2 * INTER], rhs,
                                         start=True, stop=True,
                                         tile_position=(0, 32 * i))
                    nc.vector.tensor_copy(
                        qall[:, NCHUNK * t:NCHUNK * (t + 1)], qp[:, :])

            def emit_vt(g8):
                """vt group g8: m-blocks 8*g8 .. 8*g8+7."""
                v_p = ops.tile([128, 8 * C], f32, tag="o", name="v_p")
                for jj in range(8):
                    jl = 8 * g8 + jj
                    srct = xqo if jl < NJ // 2 else xqt
                    blk = (jl % (NJ // 2)) * MBLK
                    nc.tensor.matmul(
                        v_p[:, C * jj:C * (jj + 1)],
                        srct[0:C, blk:blk + MBLK],
                        wv_t[:, :], start=True, stop=True)
                v_p8 = v_p.rearrange("p (j c) -> p j c", c=C)
                nc.vector.tensor_copy(vt3[:, 8 * g8:8 * g8 + 8, 0:C], v_p8)

            # e-tiles keyed by GLOBAL group index so HAM-warming fillers can
            # pre-touch the next group's slot.
            e_tiles = {}

            def get_e(gg):
                if gg not in e_tiles:
                    e_tiles[gg] = eps.tile([128, NCHUNK * GRP], f32,
                                           tag="e", name="e")
                return e_tiles[gg]

            def energy_mm(t, g, j, e):
                sl = slice(NCHUNK * (j - GRP * g), NCHUNK * (j - GRP * g + 1))
                i = j % 4 if ROWTILE else 0
                w = NCHUNK * (j // 4) + MBLK * (j % 4)
                if KFUSE:
                    q_rhs = kqall[32 * i + INTER:32 * i + 2 * INTER,
                                  NCHUNK * t:NCHUNK * (t + 1)]
                else:
                    q_rhs = qall[32 * i:32 * i + INTER,
                                 NCHUNK * t:NCHUNK * (t + 1)]
                nc.tensor.matmul(
                    e[:, sl],
                    kqall[32 * i:32 * i + INTER, w:w + MBLK],
                    q_rhs,
                    start=True, stop=True,
                    tile_position=(32 * i, 0) if ROWTILE else None)

            def emit_exp(t, g):
                gg = t * NGRP + g
                j0, j1 = GRP * g, min(GRP * (g + 1), NJ)
                e = get_e(gg)
                ex = wp.tile([128, NCHUNK * GRP], bf16, tag="ex", name="ex")
                nc.scalar.activation(ex[:, 0:NCHUNK * (j1 - j0)],
                                     e[:, 0:NCHUNK * (j1 - j0)], EXP)
                return ex

            def emit_av(oa, ex, g):
                j0, j1 = GRP * g, min(GRP * (g + 1), NJ)
                for j in range(j0, j1):
                    nc.tensor.matmul(oa[:, :], vt3[:, j, :],
                                     ex[:, NCHUNK * (j - j0):
                                        NCHUNK * (j - j0 + 1)],
                                     start=(j == 0), stop=(j == NJ - 1))

            def emit_epilogue(t, oa):
                # normalize + residual + store (PE-free, pipelined halves)
                nparts = 2
                HC = NCHUNK // nparts
                if t < NT - 1:
                    # copy-out first: frees oa's PSUM slot immediately so
                    # the next chunk's oa allocates without stalling, and
                    # later DVE ops read SBUF (faster access) instead.
                    oas = fp.tile([C + 1, NCHUNK], f32, tag="oas", name="oas")
                    nc.vector.tensor_copy(oas[:, :], oa[:, :])
                    src = oas
                    for hh in range(nparts):
                        hs = slice(HC * hh, HC * (hh + 1))
                        gs = slice(NCHUNK * t + HC * hh,
                                   NCHUNK * t + HC * (hh + 1))
                        rec = fp.tile([1, HC], f32, tag=f"rec{hh}", name="rec")
                        nc.vector.reciprocal(rec[:, :], src[C:C + 1, hs])
                        bcs = fp.tile([C, HC], f32, tag=f"bcs{hh}", name="bcs")
                        nc.gpsimd.partition_broadcast(bcs[:, :], rec[:, :])
                        t1 = fp.tile([C, HC], f32, tag=f"t1{hh}", name="t1")
                        nc.vector.tensor_mul(t1[:, :], src[0:C, hs], bcs[:, :])
                        fin = fp.tile([C, HC], f32, tag=f"fin{hh}", name="fin")
                        nc.vector.tensor_add(fin[:, :], t1[:, :], xr_t[:, gs])
                        nc.sync.dma_start(out=out[:, gs], in_=fin[:, :])
                else:
                    # latency-critical tail: per-half 1/x via ACT ln+exp
                    # (table set already resident; no reloads). oa is
                    # copied to SBUF once (DVE can read only one PSUM
                    # operand); the reciprocal-row broadcast runs on the
                    # idle PE (ones64^T @ rec -> PSUM) instead of the
                    # serial gpsimd queue. Phase-ordered so the ACT chain
                    # finishes first and the halves pipeline.
                    oas = fp.tile([C + 1, NCHUNK], f32, tag="oas",
                                  name="oas")
                    nc.vector.tensor_copy(oas[:, :], oa[:, :])
                    recs = []
                    for hh in range(nparts):
                        hs = slice(HC * hh, HC * (hh + 1))
                        lnt = fp.tile([1, HC], f32, tag=f"lnt{hh}", name="lnt")
                        nc.scalar.activation(lnt[:, :], oas[C:C + 1, hs],
                                             mybir.ActivationFunctionType.Ln)
                        rec = fp.tile([1, HC], bf16, tag=f"recf{hh}",
                                      name="recf")
                        nc.scalar.activation(rec[:, :], lnt[:, :], EXP,
                                             scale=-1.0)
                        recs.append(rec)
                    bc = ops.tile([C, NCHUNK], f32, tag="o", name="bc")
                    for hh in range(nparts):
                        hs = slice(HC * hh, HC * (hh + 1))
                        nc.tensor.matmul(bc[:, hs], ones64[:, :],
                                         recs[hh][:, :],
                                         start=True, stop=True)
                    for hh in range(nparts):
                        hs = slice(HC * hh, HC * (hh + 1))
                        gs = slice(NCHUNK * t + HC * hh,
                                   NCHUNK * t + HC * (hh + 1))
                        t1 = fp.tile([C, HC], f32, tag=f"t1{hh}", name="t1")
                        nc.vector.tensor_mul(t1[:, :], oas[0:C, hs],
                                             bc[:, hs])
                        fin = fp.tile([C, HC], f32, tag=f"fin{hh}", name="fin")
                        nc.vector.tensor_add(fin[:, :], t1[:, :], xr_t[:, gs])
                        nc.sync.dma_start(out=out[:, gs], in_=fin[:, :])

            # ---- emission: energies per exp-group (3 m-blocks, distinct
            # row groups -> concurrent burst); exp(g) after its energies;
            # AV lags one group so the PE can run ahead of the ACT.
            kq_done = 0
            vt_done = 0

            e_emitted = set()

            def emit_energies(gg):
                if gg in e_emitted or gg >= NT * NGRP:
                    return
                e_emitted.add(gg)
                te, ge = divmod(gg, NGRP)
                jlast = min(GRP * (ge + 1), NJ) - 1
                nonlocal kq_done
                if te == 0:
                    while kq_done <= min(jlast // 4, 2 * NT - 1):
                        emit_kq(kq_done)
                        kq_done += 1
                for j in range(GRP * ge, jlast + 1):
                    energy_mm(te, ge, j, get_e(gg))

            for t in range(NT):
                oa = ops.tile([C + 1, NCHUNK], f32, tag="o", name="oa")
                exs = {}
                for g in range(NGRP):
                    gg = t * NGRP + g
                    emit_energies(gg)
                    # one-group PE lookahead: the next group's energies go
                    # out BEFORE exp(g) / AV(g-1) so they complete inside
                    # exp(g)'s window and the ACT is never starved
                    emit_energies(gg + 1)
                    exs[g] = emit_exp(t, g)
                    if g >= 1:
                        if t == 0:
                            jprev = min(GRP * g, NJ) - 1
                            while vt_done <= min(jprev // 8, NJ // 8 - 1):
                                emit_vt(vt_done)
                                vt_done += 1
                        emit_av(oa, exs.pop(g - 1), g - 1)
                if t == 0:
                    while vt_done < NJ // 8:
                        emit_vt(vt_done)
                        vt_done += 1
                emit_av(oa, exs.pop(NGRP - 1), NGRP - 1)
                emit_epilogue(t, oa)

    nc.compile()
    return nc


def _get_compiled():
    if "nc" not in _compiled:
        _compiled["nc"] = _build()
    return _compiled["nc"]


def kernel(x, Wq, bq, Wk, bk, Wv, bv, gamma):
    global LAST_RESULT
    _ensure_ntff_hook_importable()
    from concourse.bass_utils import run_bass_kernel_spmd

    nc = _get_compiled()

    x = np.asarray(x, dtype=np.float32)
    xf = x.reshape(B, C, N)
    Wq, Wk, Wv = np.asarray(Wq), np.asarray(Wk), np.asarray(Wv)
    bq, bk, bv = np.asarray(bq), np.asarray(bk), np.asarray(bv)
    gval = float(np.asarray(gamma).reshape(-1)[0])

    # wqk [65, 16]: k weights at cols 0-7, q at cols 8-15, bias row at 64.
    wqk_a = np.zeros((C + 1, 16), np.float32)
    wqk_a[0:C, 0:INTER] = Wk.T
    wqk_a[C, 0:INTER] = bk
    wqk_a[0:C, INTER:2 * INTER] = Wq.T
    wqk_a[C, INTER:2 * INTER] = bq
    wqk_a = wqk_a.astype(ml_dtypes.bfloat16)
    wv_a = np.ascontiguousarray(gval * Wv.T).astype(ml_dtypes.bfloat16)

    in_maps = []
    for core in range(NCORES):
        b, h = divmod(core, 2)
        own = xf[b][:, h * NHALF:(h + 1) * NHALF]
        oth = xf[b][:, (1 - h) * NHALF:(2 - h) * NHALF]
        ones = np.ones((1, NHALF), dtype=np.float32)
        xbh_core = np.concatenate([own, ones, oth, ones],
                                  axis=0).astype(ml_dtypes.bfloat16)
        # gamma*bv folded into the residual (sum_m A[n,m] = denom[n])
        xres_core = own + gval * bv[:, None]
        in_maps.append({
            "xbh": np.ascontiguousarray(xbh_core),
            "xres": np.ascontiguousarray(xres_core, dtype=np.float32),
            "wqk": wqk_a, "wv_": wv_a,
        })

    trace = bool(os.environ.get("KTRACE"))
    res = run_bass_kernel_spmd(nc, in_maps, list(range(NCORES)), trace=trace)
    LAST_RESULT = res

    outf = np.empty((B, C, N), dtype=np.float32)
    for core in range(NCORES):
        b, h = divmod(core, 2)
        outf[b][:, h * NHALF:(h + 1) * NHALF] = res.results[core]["out"]
    return outf.reshape(B, C, H, W)


# revision 46
# speedup vs baseline: 1.1782x; 1.1782x over previous
"""Trainium2 Bass kernel for nn_AttentionBlock (B=4, C=64, H=W=64, INTER=8).

Sharding: 8 cores = 4 batches x 2 query-halves. Each core computes, for its
batch b and its half of the query pixels (n), the full attention output
gamma * (V @ softmax(Q^T K)^T) + x over all m=4096 keys.

SPMD uniformity trick: the host permutes each core's pixel columns so that
columns [0, 2048) are the core's OWN query half and [2048, 4096) are the
other half. Attention is permutation-invariant over keys, so every core runs
the identical program on differently-permuted data.

Design notes (the chip power-throttles under sustained load, pinning the
PE near 1.2 GHz and making the kernel PE-throughput-bound; every change
below removes PE cycles or overlaps engines):
  1. kq setup: COLUMN-TILED matmuls (4 concurrent tiles of M=8 at
     tile_position=(0,32i)) write k (and q) of each 512-pixel chunk
     directly into partition groups {0,32,64,96}+0..7 of one PSUM bank;
     a single fat DVE cast lands them in kall/qall SBUF, replacing a long
     per-group copy chain.
  2. Energy matmuls are 4-way ROW-TILED (tile_position=(32i,0), K=8 of
     32): up to 4 concurrent m-block matmuls per group burst.
  3. exp groups of GRP=3 m-blocks (3 PSUM banks x 2 bufs): 44 ACT
     instructions amortize the ~400ns/instr ACT overhead. AV matmuls lag
     one group behind exp so the PE runs ahead of the ACT.
  4. AV (vT_aug^T @ expE) accumulates out_aug[65, 512] per chunk; row 64
     (vT ones column) is the softmax denominator.
  5. gamma*bv is folded into the residual input on the host (out =
     gamma*(V@A/d) + gamma*bv + x, since sum_m A[n,m] = d[n]), so Wv has
     no bias row and vt matmuls contract over K=64.
  6. The natural_log_exp_and_others ACT table set (ln+exp) is preloaded
     once, so the tail 1/x = exp(-ln(x)) trick causes no table reloads.
     Mid-stream chunks use DVE reciprocal (slow but off the critical
     engines); their epilogue starts with a PSUM->SBUF copy-out so the
     oa bank frees immediately. The tail broadcasts the reciprocal row
     with a PE matmul (ones64^T @ rec) instead of the gpsimd queue.

Hard-won constraints (hangs/verifier): PSUM engine reads must start at a
32-aligned partition; two matmuls with DIFFERENT tile_positions must not
write the same PSUM bank (device hang); DVE tensor ops may read at most
one PSUM operand.

No max-subtraction is needed in softmax: |energy| <~ 15 for this problem's
fixed input distribution, well within fp32 exp range.
"""

import os
import sys
import types
import numpy as np
import ml_dtypes


def _ensure_ntff_hook_importable():
    """bass_utils imports antenv.axon_hooks when tracing is requested via
    BASS_TRACE; some images lack that module. Provide it (backed by the
    ctypes hook from trn_boot when available, else a None hook, which
    bass_utils handles by skipping the trace)."""
    try:
        import antenv.axon_hooks  # noqa: F401
        return
    except ImportError:
        pass
    hook = None
    try:
        from trn_agent_boot.trn_boot import _ntff_profile_via_ctypes
        so = "/opt/axon/libaxon_pjrt.so"
        if os.path.exists(so):
            hook = _ntff_profile_via_ctypes(so)
    except Exception:
        hook = None
    mod = types.ModuleType("antenv.axon_hooks")
    mod.get_axon_ntff_profile_hook = lambda: hook
    sys.modules["antenv.axon_hooks"] = mod

B, C, H, W = 4, 64, 64, 64
N = H * W              # 4096 pixels
NHALF = N // 2         # 2048 query pixels per core
INTER = C // 8         # 8
NCORES = 8
MBLK = 128             # m-block (PSUM partition tile)
NCHUNK = 512           # query-chunk (PSUM bank free size)
NJ = N // MBLK         # 32 m-blocks
NT = NHALF // NCHUNK   # 4 query chunks
GRP = int(os.environ.get("KGRP", "3"))      # m-blocks per exp instruction
NGRP = (NJ + GRP - 1) // GRP                # exp groups per chunk
ROWTILE = int(os.environ.get("KROWTILE", "1"))  # 4-way PE row tiling
KFUSE = int(os.environ.get("KFUSE", "0"))   # fused k+q bursts: rejected by
# the BIR verifier (matmul operands must start 32-partition-aligned, and
# fusing puts q at base 32i+8); kept for reference
ACT_SET_LN_EXP = 6     # act_info.json index of natural_log_exp_and_others

_compiled = {}
LAST_RESULT = None


def _build():
    import concourse.bacc as bacc
    import concourse.mybir as mybir
    from concourse.tile import TileContext

    dt = mybir.dt
    f32, bf16 = dt.float32, dt.bfloat16
    EXP = mybir.ActivationFunctionType.Exp
    LN = mybir.ActivationFunctionType.Ln

    nc = bacc.Bacc("TRN2", target_bir_lowering=False, debug=False,
                   num_devices=NCORES)

    # host-prepped inputs (see kernel() below)
    xbh = nc.dram_tensor("xbh", [130, NHALF], bf16, kind="ExternalInput").ap()
    xres = nc.dram_tensor("xres", [C, NHALF], f32, kind="ExternalInput").ap()
    wqk = nc.dram_tensor("wqk", [C + 1, 16], bf16, kind="ExternalInput").ap()
    wv = nc.dram_tensor("wv_", [C, C], bf16, kind="ExternalInput").ap()
    out = nc.dram_tensor("out", [C, NHALF], f32, kind="ExternalOutput").ap()

    with TileContext(nc) as tc:
        with tc.tile_pool(name="const", bufs=1) as cp, \
             tc.tile_pool(name="eps", bufs=2, space="PSUM") as eps, \
             tc.tile_pool(name="ops", bufs=8 - 2 * GRP,
                          space="PSUM") as ops, \
             tc.tile_pool(name="work", bufs=3) as wp, \
             tc.tile_pool(name="fin", bufs=2) as fp:

            # Preload the ln+exp activation-table set so neither the exp
            # stream nor the tail 1/x = exp(-ln x) forces a table switch.
            nc.scalar.add_instruction(mybir.InstLoadActFuncSet(
                name=nc.scalar.bass.get_next_instruction_name(),
                act_func_set_id=ACT_SET_LN_EXP, ins=[], outs=[]))

            # DMA issue order matters: the first kq matmul needs xqo piece
            # one + wqk; wv is needed by the first vt group shortly after;
            # xres only by the chunk-0 epilogue.
            wqk_t = cp.tile([C + 1, 16], bf16, tag="wqk", name="wqk_t")
            nc.sync.dma_start(out=wqk_t[:, :], in_=wqk)
            xqo = cp.tile([C + 1, NHALF], bf16, tag="xqo", name="xqo")
            nc.sync.dma_start(out=xqo[:, 0:NCHUNK], in_=xbh[0:C + 1, 0:NCHUNK])
            wv_t = cp.tile([C, C], bf16, tag="wv", name="wv_t")
            nc.sync.dma_start(out=wv_t[:, :], in_=wv)
            nc.sync.dma_start(out=xqo[:, NCHUNK:], in_=xbh[0:C + 1, NCHUNK:])
            xqt = cp.tile([C + 1, NHALF], bf16, tag="xqt", name="xqt")
            nc.sync.dma_start(out=xqt[:, :], in_=xbh[C + 1:2 * C + 2, :])
            xr_t = cp.tile([C, NHALF], f32, tag="xr", name="xr_t")
            nc.sync.dma_start(out=xr_t[:, :], in_=xres)
            ones64 = cp.tile([1, C], bf16, tag="ones64", name="ones64")
            nc.vector.memset(ones64[:, :], 1.0)

            # kall/qall: k (q) of every 512-pixel chunk replicated across
            # partition groups {0,32,64,96}+0..7 by COLUMN-TILED kq matmuls
            # (4 concurrent tiles, tile_position=(0,32i)), so a single fat
            # PSUM->SBUF cast replaces the whole per-group copy chain.
            # Energy lhsT for m-block j: kall[32(j%4)+0..8,
            # 512(j//4)+128(j%4) ..+128]; rhs: qall[32i+0..8, chunk].
            # kqall: per 512-pixel chunk window, partitions 32i+0..7 hold
            # that chunk's k and 32i+8..15 its q (all four groups), written
            # by ONE fused column-tiled burst (M=16 at tile_position
            # (0,32i)) and ONE fat cast. With KFUSE=0, separate k/q tiles.
            kqall = cp.tile([128, N], bf16, tag="kqall", name="kqall")
            if not KFUSE:
                qall = cp.tile([128, NHALF], bf16, tag="qall", name="qall")
            vt = cp.tile([128, NJ * (C + 1)], bf16, tag="vt", name="vt")
            vt3 = vt.rearrange("p (j c) -> p j c", c=C + 1)
            nc.vector.memset(vt3[:, :, C], 1.0)

            def emit_kq(kc):
                """kq chunk kc (0-3 own half, 4-7 other half)."""
                own = kc < NT
                srct = xqo if own else xqt
                t = kc % NT
                rhs = srct[:, NCHUNK * t:NCHUNK * (t + 1)]
                kp = ops.tile([128, NCHUNK], f32, tag="o", name="kp")
                m = 2 * INTER if KFUSE else INTER
                for i in range(4):
                    nc.tensor.matmul(kp[32 * i:32 * i + m, :],
                                     wqk_t[:, 0:m], rhs,
                                     start=True, stop=True,
                                     tile_position=(0, 32 * i))
                nc.vector.tensor_copy(
                    kqall[:, NCHUNK * kc:NCHUNK * (kc + 1)], kp[:, :])
                if own and not KFUSE:
                    qp = ops.tile([128, NCHUNK], f32, tag="o", name="qp")
                    for i in range(4):
                        nc.tensor.matmul(qp[32 * i:32 * i + INTER, :],
                                         wqk_t[:, INTER:2 * INTER], rhs,
                                         start=True, stop=True,
                                         tile_position=(0, 32 * i))
                    nc.vector.tensor_copy(
                        qall[:, NCHUNK * t:NCHUNK * (t + 1)], qp[:, :])

            def emit_vt(g8):
                """vt group g8: m-blocks 8*g8 .. 8*g8+7."""
                v_p = ops.tile([128, 8 * C], f32, tag="o", name="v_p")
                for jj in range(8):
                    jl = 8 * g8 + jj
                    srct = xqo if jl < NJ // 2 else xqt
                    blk = (jl % (NJ // 2)) * MBLK
                    nc.tensor.matmul(
                        v_p[:, C * jj:C * (jj + 1)],
                        srct[0:C, blk:blk + MBLK],
                        wv_t[:, :], start=True, stop=True)
                v_p8 = v_p.rearrange("p (j c) -> p j c", c=C)
                nc.vector.tensor_copy(vt3[:, 8 * g8:8 * g8 + 8, 0:C], v_p8)

            # e-tiles keyed by GLOBAL group index so HAM-warming fillers can
            # pre-touch the next group's slot.
            e_tiles = {}

            def get_e(gg):
                if gg not in e_tiles:
                    e_tiles[gg] = eps.tile([128, NCHUNK * GRP], f32,
                                           tag="e", name="e")
                return e_tiles[gg]

            def energy_mm(t, g, j, e):
                sl = slice(NCHUNK * (j - GRP * g), NCHUNK * (j - GRP * g + 1))
                i = j % 4 if ROWTILE else 0
                w = NCHUNK * (j // 4) + MBLK * (j % 4)
                if KFUSE:
                    q_rhs = kqall[32 * i + INTER:32 * i + 2 * INTER,
                                  NCHUNK * t:NCHUNK * (t + 1)]
                else:
                    q_rhs = qall[32 * i:32 * i + INTER,
                                 NCHUNK * t:NCHUNK * (t + 1)]
                nc.tensor.matmul(
                    e[:, sl],
                    kqall[32 * i:32 * i + INTER, w:w + MBLK],
                    q_rhs,
                    start=True, stop=True,
                    tile_position=(32 * i, 0) if ROWTILE else None)

            def emit_exp(t, g):
                gg = t * NGRP + g
                j0, j1 = GRP * g, min(GRP * (g + 1), NJ)
                e = get_e(gg)
                ex = wp.tile([128, NCHUNK * GRP], bf16, tag="ex", name="ex")
                nc.scalar.activation(ex[:, 0:NCHUNK * (j1 - j0)],
                                     e[:, 0:NCHUNK * (j1 - j0)], EXP)
                return ex

            def emit_av(oa, ex, g):
                j0, j1 = GRP * g, min(GRP * (g + 1), NJ)
                for j in range(j0, j1):
                    nc.tensor.matmul(oa[:, :], vt3[:, j, :],
                                     ex[:, NCHUNK * (j - j0):
                                        NCHUNK * (j - j0 + 1)],
                                     start=(j == 0), stop=(j == NJ - 1))

            def emit_epilogue(t, oa):
                # normalize + residual + store (PE-free, pipelined halves)
                nparts = 2
                HC = NCHUNK // nparts
                if t < NT - 1:
                    # copy-out first: frees oa's PSUM slot immediately so
                    # the next chunk's oa allocates without stalling, and
                    # later DVE ops read SBUF (faster access) instead.
                    oas = fp.tile([C + 1, NCHUNK], f32, tag="oas", name="oas")
                    nc.vector.tensor_copy(oas[:, :], oa[:, :])
                    src = oas
                    for hh in range(nparts):
                        hs = slice(HC * hh, HC * (hh + 1))
                        gs = slice(NCHUNK * t + HC * hh,
                                   NCHUNK * t + HC * (hh + 1))
                        rec = fp.tile([1, HC], f32, tag=f"rec{hh}", name="rec")
                        nc.vector.reciprocal(rec[:, :], src[C:C + 1, hs])
                        bcs = fp.tile([C, HC], f32, tag=f"bcs{hh}", name="bcs")
                        nc.gpsimd.partition_broadcast(bcs[:, :], rec[:, :])
                        t1 = fp.tile([C, HC], f32, tag=f"t1{hh}", name="t1")
                        nc.vector.tensor_mul(t1[:, :], src[0:C, hs], bcs[:, :])
                        fin = fp.tile([C, HC], f32, tag=f"fin{hh}", name="fin")
                        nc.vector.tensor_add(fin[:, :], t1[:, :], xr_t[:, gs])
                        nc.sync.dma_start(out=out[:, gs], in_=fin[:, :])
                else:
                    # latency-critical tail: per-half 1/x via ACT ln+exp
                    # (table set already resident; no reloads). oa is
                    # copied to SBUF once (DVE can read only one PSUM
                    # operand); the reciprocal-row broadcast runs on the
                    # idle PE (ones64^T @ rec -> PSUM) instead of the
                    # serial gpsimd queue. Phase-ordered so the ACT chain
                    # finishes first and the halves pipeline.
                    oas = fp.tile([C + 1, NCHUNK], f32, tag="oas",
                                  name="oas")
                    nc.vector.tensor_copy(oas[:, :], oa[:, :])
                    recs = []
                    for hh in range(nparts):
                        hs = slice(HC * hh, HC * (hh + 1))
                        lnt = fp.tile([1, HC], f32, tag=f"lnt{hh}", name="lnt")
                        nc.scalar.activation(lnt[:, :], oas[C:C + 1, hs],
                                             mybir.ActivationFunctionType.Ln)
                        rec = fp.tile([1, HC], bf16, tag=f"recf{hh}",
                                      name="recf")
                        nc.scalar.activation(rec[:, :], lnt[:, :], EXP,
                                             scale=-1.0)
                        recs.append(rec)
                    bc = ops.tile([C, NCHUNK], f32, tag="o", name="bc")
                    for hh in range(nparts):
                        hs = slice(HC * hh, HC * (hh + 1))
                        nc.tensor.matmul(bc[:, hs], ones64[:, :],
                                         recs[hh][:, :],
                                         start=True, stop=True)
                    for hh in range(nparts):
                        hs = slice(HC * hh, HC * (hh + 1))
                        gs = slice(NCHUNK * t + HC * hh,
                                   NCHUNK * t + HC * (hh + 1))
                        t1 = fp.tile([C, HC], f32, tag=f"t1{hh}", name="t1")
                        nc.vector.tensor_mul(t1[:, :], oas[0:C, hs],
                                             bc[:, hs])
                        fin = fp.tile([C, HC], f32, tag=f"fin{hh}", name="fin")
                        nc.vector.tensor_add(fin[:, :], t1[:, :], xr_t[:, gs])
                        nc.sync.dma_start(out=out[:, gs], in_=fin[:, :])

            # ---- emission: energies per exp-group (3 m-blocks, distinct
            # row groups -> concurrent burst); exp(g) after its energies;
            # AV lags one group so the PE can run ahead of the ACT.
            kq_done = 0
            vt_done = 0

            e_emitted = set()

            def emit_energies(gg):
                if gg in e_emitted or gg >= NT * NGRP:
                    return
                e_emitted.add(gg)
                te, ge = divmod(gg, NGRP)
                jlast = min(GRP * (ge + 1), NJ) - 1
                nonlocal kq_done
                if te == 0:
                    while kq_done <= min(jlast // 4, 2 * NT - 1):
                        emit_kq(kq_done)
                        kq_done += 1
                for j in range(GRP * ge, jlast + 1):
                    energy_mm(te, ge, j, get_e(gg))

            for t in range(NT):
                oa = ops.tile([C + 1, NCHUNK], f32, tag="o", name="oa")
                exs = {}
                for g in range(NGRP):
                    gg = t * NGRP + g
                    emit_energies(gg)
                    # one-group PE lookahead: the next group's energies go
                    # out BEFORE exp(g) / AV(g-1) so they complete inside
                    # exp(g)'s window and the ACT is never starved
                    emit_energies(gg + 1)
                    exs[g] = emit_exp(t, g)
                    if g >= 1:
                        if t == 0:
                            jprev = min(GRP * g, NJ) - 1
                            while vt_done <= min(jprev // 8, NJ // 8 - 1):
                                emit_vt(vt_done)
                                vt_done += 1
                        emit_av(oa, exs.pop(g - 1), g - 1)
                if t == 0:
                    while vt_done < NJ // 8:
                        emit_vt(vt_done)
                        vt_done += 1
                emit_av(oa, exs.pop(NGRP - 1), NGRP - 1)
                emit_epilogue(t, oa)

    nc.compile()
    return nc


def _get_compiled():
    if "nc" not in _compiled:
        _compiled["nc"] = _build()
    return _compiled["nc"]


def kernel(x, Wq, bq, Wk, bk, Wv, bv, gamma):
    global LAST_RESULT
    _ensure_ntff_hook_importable()
    from concourse.bass_utils import run_bass_kernel_spmd

    nc = _get_compiled()

    x = np.asarray(x, dtype=np.float32)
    xf = x.reshape(B, C, N)
    Wq, Wk, Wv = np.asarray(Wq), np.asarray(Wk), np.asarray(Wv)
    bq, bk, bv = np.asarray(bq), np.asarray(bk), np.asarray(bv)
    gval = float(np.asarray(gamma).reshape(-1)[0])

    # wqk [65, 16]: k weights at cols 0-7, q at cols 8-15, bias row at 64.
    wqk_a = np.zeros((C + 1, 16), np.float32)
    wqk_a[0:C, 0:INTER] = Wk.T
    wqk_a[C, 0:INTER] = bk
    wqk_a[0:C, INTER:2 * INTER] = Wq.T
    wqk_a[C, INTER:2 * INTER] = bq
    wqk_a = wqk_a.astype(ml_dtypes.bfloat16)
    wv_a = np.ascontiguousarray(gval * Wv.T).astype(ml_dtypes.bfloat16)

    in_maps = []
    for core in range(NCORES):
        b, h = divmod(core, 2)
        own = xf[b][:, h * NHALF:(h + 1) * NHALF]
        oth = xf[b][:, (1 - h) * NHALF:(2 - h) * NHALF]
        ones = np.ones((1, NHALF), dtype=np.float32)
        xbh_core = np.concatenate([own, ones, oth, ones],
                                  axis=0).astype(ml_dtypes.bfloat16)
        # gamma*bv folded into the residual (sum_m A[n,m] = denom[n])
        xres_core = own + gval * bv[:, None]
        in_maps.append({
            "xbh": np.ascontiguousarray(xbh_core),
            "xres": np.ascontiguousarray(xres_core, dtype=np.float32),
            "wqk": wqk_a, "wv_": wv_a,
        })

    trace = bool(os.environ.get("KTRACE"))
    res = run_bass_kernel_spmd(nc, in_maps, list(range(NCORES)), trace=trace)
    LAST_RESULT = res

    outf = np.empty((B, C, N), dtype=np.float32)
    for core in range(NCORES):
        b, h = divmod(core, 2)
        outf[b][:, h * NHALF:(h + 1) * NHALF] = res.results[core]["out"]
    return outf.reshape(B, C, H, W)
